# revision 19
# baseline (speedup 1.0000x reference)
"""Trainium2 Bass kernel for a biased transformer encoder layer.

Full (unsharded) inputs -> full output. Data-parallel over batch B across 8
NeuronCores (one batch element per core).

v2 design points (on top of the v1 identity-injection design):
- Single ACT table set (`exp_and_others`): only Exp / Relu / Copy / Identity
  run on ScalarE.  LayerNorm rsqrt is a batched DVE bit-trick + 2 Newton
  steps; softmax denominators use DVE reciprocal_approx_fast per pair.
  This removes all mid-kernel ACT_TABLE_LOADs and the serial denominator
  stall that let the PE go HAM-cold before the FFN phase.
- exp() stays at [128,1024] per call: with the 3-slot PSUM score ring the
  next iteration's bias-injection reuses the slot freed by the FIRST of
  the two exp calls, so finer exp granularity is what keeps the PE from
  stalling on ScalarE.
- s/t are laid out as seq = 8*partition + tile so src load and output
  store are single DMAs with 8KB/4KB contiguous per-partition lines
  (the bias host-pack absorbs the permutation).
- Output is stored per s-half on the second HWDGE queue (scalar engine)
  so the store overlaps the FFN of the second half.
- PSUM evacuations (x^T, q/k^T, y^T, f2) and the FFN relu run on ScalarE
  (Copy/Relu are in the exp table set); LayerNorm stats/apply stay on DVE.
"""

import numpy as np
import ml_dtypes

import concourse.mybir as mybir
import concourse.tile as tile
from concourse import bacc
from concourse.bass_utils import run_bass_kernel_spmd

# ---- problem constants (hardcoded per contract) ----
S = 1024
B = 8
D = 256
H = 8
HD = D // H          # 32
DFF = 1024
EPS = 1e-5
N_CORES = 8
NT = S // 128        # 8 s-tiles / t-tiles

F32 = mybir.dt.float32
BF16 = mybir.dt.bfloat16
I32 = mybir.dt.int32
bf16 = ml_dtypes.bfloat16
AF = mybir.ActivationFunctionType
ALU = mybir.AluOpType

# weight-pack column offsets (shared by _build and _prep_host)
OFF_ID = 0                      # identity          [128, 128]
OFF_QK = OFF_ID + 128           # wqkT  2x[128,512]
OFF_WV = OFF_QK + 1024          # wv dense 2x[128,256]
OFF_WO = OFF_WV + 512           # woPK  4x[128,256] (zero-padded pair blocks)
OFF_W1 = OFF_WO + 1024          # w1T   2x[128,1024]
OFF_W2 = OFF_W1 + 2048          # w2T   8x[128,256]
WPK_COLS = OFF_W2 + 2048        # 6784
WPK_A = OFF_W1                  # phase-1/2 weights (first DMA chunk)

RSQRT_MAGIC1 = 0x5F3759DF + 1   # seed = (~(i>>1)) + MAGIC+1 == MAGIC - (i>>1)

_CACHE = {}


def _install_axon_hooks_shim():
    """Make `trace=True` degrade gracefully if antenv.axon_hooks is missing."""
    import sys, types
    try:
        import antenv  # noqa
    except ImportError:
        return
    if "antenv.axon_hooks" in sys.modules:
        return
    try:
        import antenv.axon_hooks  # noqa
    except ImportError:
        import antenv
        mod = types.ModuleType("antenv.axon_hooks")
        _hook = [None]
        mod.set_axon_ntff_profile_hook = lambda h: _hook.__setitem__(0, h)
        mod.get_axon_ntff_profile_hook = lambda: _hook[0]
        sys.modules["antenv.axon_hooks"] = mod
        antenv.axon_hooks = mod


def _build(flags):
    """Build the Bass program (shared by all 8 cores, SPMD)."""
    (aff1, aff2, has_bqk, has_bo, has_b1, has_b2, has_bv) = flags
    nc = bacc.Bacc("TRN2", debug=False, num_devices=N_CORES, enable_asserts=True)

    # ---- DRAM tensors (per-core inputs) ----
    src_d = nc.dram_tensor("src", [S, D], F32, kind="ExternalInput")
    expb_d = nc.dram_tensor("biasPK", [4, 8, 128, 2048], BF16, kind="ExternalInput")
    wpk_d = nc.dram_tensor("wpk", [128, WPK_COLS], BF16, kind="ExternalInput")
    if has_bv:
        bvb_d = nc.dram_tensor("bvb", [128, D], F32, kind="ExternalInput")
    if has_bqk:
        bqk_d = nc.dram_tensor("bqk", [128, 4], F32, kind="ExternalInput")
    if aff1:
        g1b_d = nc.dram_tensor("g1b", [128, D], F32, kind="ExternalInput")
        be1b_d = nc.dram_tensor("be1b", [128, D], F32, kind="ExternalInput")
    if aff2:
        g2b_d = nc.dram_tensor("g2b", [128, D], F32, kind="ExternalInput")
        be2b_d = nc.dram_tensor("be2b", [128, D], F32, kind="ExternalInput")
    if has_bo:
        bob_d = nc.dram_tensor("bob", [128, D], F32, kind="ExternalInput")
    if has_b1:
        b1c_d = nc.dram_tensor("b1c", [128, DFF // 128], F32, kind="ExternalInput")
    if has_b2:
        b2c_d = nc.dram_tensor("b2c", [128, D // 128], F32, kind="ExternalInput")
    out_d = nc.dram_tensor("out", [S, D], F32, kind="ExternalOutput")
    import os
    dbg = os.environ.get("KBG_DEBUG", "")
    dbg_d = {}
    if dbg:
        for nm, shape, dt in [
                ("dbg_xbf0", [128, D], BF16), ("dbg_xnT0", [128, S], BF16),
                ("dbg_qT0", [128, 512], BF16), ("dbg_kT0", [128, 512], BF16),
                ("dbg_vx0", [128, 2 * D], BF16), ("dbg_pts0", [128, 2048], BF16),
                ("dbg_ctxU0", [128, S], F32), ("dbg_ctxN0", [128, S], BF16),
                ("dbg_ybf0", [128, D], BF16), ("dbg_rstd1", [128, 2 * NT], F32),
                ("dbg_f1T0", [128, S], BF16), ("dbg_recU", [128, S], F32), ("dbg_recV", [128, S], F32)]:
            dbg_d[nm] = nc.dram_tensor(nm, shape, dt, kind="ExternalOutput")

    with tile.TileContext(nc, trace_sim=True) as tc:
        with tc.tile_pool(name="persist", bufs=1) as pp, \
             tc.tile_pool(name="battn", bufs=16) as battn:
            wpk = pp.tile([128, WPK_COLS], BF16, tag="wpk", name="wpk")
            identb = wpk[:, OFF_ID:OFF_ID + 128]

            if has_bv:
                bvb = pp.tile([128, D], F32, tag="bvb", name="bvb")
            if has_bqk:
                bqk = pp.tile([128, 4], F32, tag="bqk", name="bqk")
            if aff1:
                g1b = pp.tile([128, D], F32, tag="g1b", name="g1b")
                be1b = pp.tile([128, D], F32, tag="be1b", name="be1b")
            if aff2:
                g2b = pp.tile([128, D], F32, tag="g2b", name="g2b")
                be2b = pp.tile([128, D], F32, tag="be2b", name="be2b")
            if has_bo:
                bob = pp.tile([128, D], F32, tag="bob", name="bob")
            if has_b1:
                b1c = pp.tile([128, DFF // 128], F32, tag="b1c", name="b1c")
            if has_b2:
                b2c = pp.tile([128, D // 128], F32, tag="b2c", name="b2c")

            # ---- persistent activations ----
            srcall = pp.tile([128, 8 * D], F32, tag="srcall", name="srcall")
            xbf = [pp.tile([128, D], BF16, tag=f"xbf{i}", name=f"xbf{i}") for i in range(NT)]
            x_res = xbf
            if aff1:
                xn = [pp.tile([128, D], F32, tag=f"xn{i}", name=f"xn{i}") for i in range(NT)]
                x_res = [pp.tile([128, D], F32, tag=f"xr{i}", name=f"xr{i}") for i in range(NT)]
            xnT = [pp.tile([128, S], BF16, tag=f"xnT{k}", name=f"xnT{k}") for k in range(2)]
            qT = [pp.tile([128, 512], BF16, tag=f"qT{k}", name=f"qT{k}") for k in range(4)]
            kT = [pp.tile([128, 512], BF16, tag=f"kT{k}", name=f"kT{k}") for k in range(4)]
            vx = [pp.tile([128, 2 * D], BF16, tag=f"vx{i}", name=f"vx{i}") for i in range(NT)]
            pts = pp.tile([128, 4096], BF16, tag="pts", name="pts")
            ctxU = [pp.tile([128, S], F32, tag=f"ctxU{p}", name=f"ctxU{p}") for p in range(4)]
            ctxN = [pp.tile([128, S], BF16, tag=f"ctxN{p}", name=f"ctxN{p}") for p in range(4)]
            recU = pp.tile([128, S], F32, tag="recU", name="recU")
            recV = pp.tile([128, S], F32, tag="recV", name="recV")
            ybf = [pp.tile([128, D], BF16, tag=f"ybf{i}", name=f"ybf{i}") for i in range(NT)]
            y_res = ybf
            if aff2:
                yn = [pp.tile([128, D], F32, tag=f"yn{i}", name=f"yn{i}") for i in range(NT)]
                y_res = [pp.tile([128, D], F32, tag=f"yr{i}", name=f"yr{i}") for i in range(NT)]
            ynT = [pp.tile([128, S], BF16, tag=f"ynT{k}", name=f"ynT{k}") for k in range(2)]
            f1T = [pp.tile([128, S], BF16, tag=f"f1T{m}", name=f"f1T{m}") for m in range(8)]
            f2T = [pp.tile([128, S], BF16, tag=f"f2T{m}", name=f"f2T{m}") for m in range(2)]
            outbuf = pp.tile([128, 8 * D], F32, tag="outbuf", name="outbuf")

            # rsqrt scratch (batched over tiles; [mean,var] interleaved flat —
            # the chain runs over both columns; mean lanes produce garbage
            # that is never consumed)
            aggr1 = pp.tile([128, 2 * NT], F32, tag="aggr1", name="aggr1")
            aggr2 = pp.tile([128, 2 * NT], F32, tag="aggr2", name="aggr2")
            rstd1 = pp.tile([128, 2 * NT], F32, tag="rstd1", name="rstd1")
            rstd2 = pp.tile([128, 2 * NT], F32, tag="rstd2", name="rstd2")
            rs_v = pp.tile([128, 2 * NT], F32, tag="rs_v", name="rs_v")
            rs_sh = pp.tile([128, 2 * NT], I32, tag="rs_sh", name="rs_sh")
            rs_y = pp.tile([128, 2 * NT], F32, tag="rs_y", name="rs_y")
            rs_a = pp.tile([128, 2 * NT], F32, tag="rs_a", name="rs_a")
            rs_b = pp.tile([128, 2 * NT], F32, tag="rs_b", name="rs_b")
            rs_y2 = pp.tile([128, 2 * NT], F32, tag="rs_y2", name="rs_y2")
            dum = pp.tile([128, 1], F32, tag="dum", name="dum")

            # table preload: dummy exp is the first ScalarE activation, so the
            # single exp_and_others ACT_TABLE_LOAD overlaps the initial DMAs.
            nc.gpsimd.memset(dum[:], 0.0)
            nc.scalar.activation(dum[:], dum[:], AF.Exp)

            # vx: memset to 1.0; v columns overwritten later (surviving 1.0
            # columns become the softmax-denominator "ones" blocks).
            for i in range(NT):
                nc.gpsimd.memset(vx[i][:], 1.0)
            # ctxN den rows must be 0.0 so the (zero-padded) out-proj blocks
            # never meet NaN garbage; v rows are overwritten per pair.
            for p in range(4):
                nc.gpsimd.memset(ctxN[p][:], 0.0)

            # ---- DMAs: src first (gates phase 1), then phase-1/2 weights,
            # then the full bias stream (pool-gated), FFN weights on the
            # second HWDGE queue so they don't block the bias stream.
            nc.sync.dma_start(
                srcall[:].rearrange("p (i d) -> p i d", i=NT),
                src_d.ap().rearrange("(p i) d -> p i d", i=NT))
            nc.sync.dma_start(wpk[:, 0:WPK_A], wpk_d.ap()[:, 0:WPK_A])
            nc.scalar.dma_start(wpk[:, WPK_A:WPK_COLS], wpk_d.ap()[:, WPK_A:WPK_COLS])
            if has_bv:
                nc.scalar.dma_start(bvb[:], bvb_d.ap())
            if has_bqk:
                nc.scalar.dma_start(bqk[:], bqk_d.ap())
            if aff1:
                nc.scalar.dma_start(g1b[:], g1b_d.ap())
                nc.scalar.dma_start(be1b[:], be1b_d.ap())
            if aff2:
                nc.scalar.dma_start(g2b[:], g2b_d.ap())
                nc.scalar.dma_start(be2b[:], be2b_d.ap())
            if has_bo:
                nc.scalar.dma_start(bob[:], bob_d.ap())
            if has_b1:
                nc.scalar.dma_start(b1c[:], b1c_d.ap())
            if has_b2:
                nc.scalar.dma_start(b2c[:], b2c_d.ap())
            ebs = {}
            for p in range(4):
                for t in range(NT):
                    eb = battn.tile([128, 2048], BF16, tag="eb", name="eb")
                    nc.sync.dma_start(eb[:], expb_d.ap()[p, t])
                    ebs[(p, t)] = eb

            def rsqrt_batch(aggr, rstd_out, c0, n2):
                """rstd_out[:, c] = 1/sqrt(aggr[:, c] + EPS) for c in
                [c0, c0+n2).  Runs over [mean,var] interleaved columns; only
                odd (var) lanes are consumed.  Quake-III seed + 2 Newton
                steps, all on DVE (no ACT tables)."""
                sl = (slice(None), slice(c0, c0 + n2))
                v = rs_v[sl]
                nc.vector.tensor_scalar_add(v, aggr[sl], EPS)
                nc.vector.tensor_scalar(
                    rs_sh[sl], v.bitcast(I32), 1, -1,
                    ALU.logical_shift_right, ALU.bitwise_xor)
                nc.vector.tensor_scalar_add(
                    rs_y[sl].bitcast(I32), rs_sh[sl], RSQRT_MAGIC1)
                y = rs_y[sl]
                for it in range(2):
                    yo = (rs_y2 if it == 0 else rstd_out)[sl]
                    nc.vector.tensor_tensor(rs_a[sl], y, y, ALU.mult)
                    nc.vector.tensor_tensor(rs_b[sl], rs_a[sl], v, ALU.mult)
                    nc.vector.tensor_scalar(
                        rs_a[sl], rs_b[sl], -0.5, 1.5, ALU.mult, ALU.add)
                    nc.vector.tensor_tensor(yo, rs_a[sl], y, ALU.mult)
                    y = yo

            # ================= Phase 1: LN1, transposes, v, qkT =============
            with tc.tile_pool(name="work1", bufs=4) as wk, \
                 tc.tile_pool(name="ps1", bufs=1, space="PSUM") as ps1:
                pvall = ps1.tile([128, 8 * D], F32, tag="pvall", name="pvall")

                def ln1_apply(i):
                    st = srcall[:, D * i:D * (i + 1)]
                    mean = aggr1[:, 2 * i:2 * i + 1]
                    rstd = rstd1[:, 2 * i + 1:2 * i + 2]
                    if aff1:
                        nc.vector.tensor_scalar(
                            xn[i][:], st, mean, rstd, ALU.subtract, ALU.mult)
                        nc.gpsimd.tensor_copy(xbf[i][:], xn[i][:])
                        tmp = wk.tile([128, D], F32, tag="afftmp", name="afftmp")
                        nc.vector.tensor_tensor(tmp[:], xn[i][:], g1b[:], ALU.mult)
                        nc.vector.tensor_tensor(x_res[i][:], tmp[:], be1b[:], ALU.add)
                    else:
                        nc.vector.tensor_scalar(
                            xbf[i][:], st, mean, rstd, ALU.subtract, ALU.mult)

                # batched LN1 stats per s-half, then apply + transpose + v-proj
                for half in range(2):
                    i0 = 4 * half
                    for i in range(i0, i0 + 4):
                        st = srcall[:, D * i:D * (i + 1)]
                        stats = wk.tile([128, 6], F32, tag="lnstats", name="lnstats")
                        nc.vector.bn_stats(stats[:], st)
                        nc.vector.bn_aggr(aggr1[:, 2 * i:2 * i + 2], stats[:])
                    rsqrt_batch(aggr1, rstd1, 2 * i0, 8)
                    for i in range(i0, i0 + 4):
                        ln1_apply(i)
                        # transpose s-tile into xnT columns (both d-blocks)
                        tp = ps1.tile([128, 256], BF16, tag="tp", name="tp", bufs=2)
                        for j in range(2):
                            nc.tensor.transpose(
                                tp[:, 128 * j:128 * (j + 1)],
                                xbf[i][:, 128 * j:128 * (j + 1)], identb)
                        for j in range(2):
                            nc.scalar.copy(
                                xnT[j][:, 128 * i:128 * (i + 1)],
                                tp[:, 128 * j:128 * (j + 1)])
                        # dense v projection for this tile
                        pv = pvall[:, D * i:D * (i + 1)]
                        for k in range(2):
                            nc.tensor.matmul(
                                pv,
                                xnT[k][:, 128 * i:128 * (i + 1)],
                                wpk[:, OFF_WV + 256 * k:OFF_WV + 256 * (k + 1)],
                                start=(k == 0), stop=(k == 1))
                        vdst = vx[i][:].rearrange("p (h c) -> p h c", h=H)[:, :, 0:HD]
                        vsrc = pv.rearrange("p (h c) -> p h c", h=H)
                        if has_bv:
                            bsrc = bvb[:].rearrange("p (h c) -> p h c", h=H)
                            nc.vector.tensor_tensor(vdst, vsrc, bsrc, ALU.add)
                        else:
                            nc.vector.tensor_copy(vdst, vsrc)
                    # qkT for the completed s-half
                    for m in (0, 2, 1, 3):  # q0,k0 first
                        dstT = (qT[2 * m + half] if m < 2
                                else kT[2 * (m - 2) + half])
                        pq = ps1.tile([128, 512], F32, tag="pq", name="pq", bufs=2)
                        for k in range(2):
                            nc.tensor.matmul(
                                pq[:],
                                wpk[:, OFF_QK + 512 * k + 128 * m:
                                    OFF_QK + 512 * k + 128 * (m + 1)],
                                xnT[k][:, 512 * half:512 * (half + 1)],
                                start=(k == 0), stop=(k == 1))
                        if has_bqk:
                            nc.vector.tensor_scalar_add(dstT[:], pq[:], bqk[:, m:m + 1])
                        elif m < 2:
                            nc.scalar.copy(dstT[:], pq[:])
                        else:
                            nc.vector.tensor_copy(dstT[:], pq[:])

            # ================= Phase 2: attention =================
            # Per (pair, t): raw bias is injected into PSUM by identity
            # matmuls (start=True), QK^T accumulates on top (row-tiled bands,
            # concurrent); exp spans both 1024-col halves in one ScalarE call
            # when the rotation slots are contiguous.  PV is emitted with one
            # iteration of lag so the PE FIFO never blocks on exp.
            with tc.tile_pool(name="ps2", bufs=1, space="PSUM") as ps2:
                scs3 = ps2.tile([128, 3072], F32, tag="scs3", name="scs3")
                prev = None   # deferred PV emission (cross-iteration lag)

                def pv_emit(st):
                    ctx_, pp_, tt, X = st
                    st0, sp0 = (tt == 0), (tt == NT - 1)
                    h0, h1 = 2 * pp_, 2 * pp_ + 1
                    for half in range(2):
                        psl = pts[:, X + 1024 * half:X + 1024 * (half + 1)]
                        nc.tensor.matmul(
                            ctx_[0:64, 512 * half:512 * (half + 1)],
                            vx[tt][:, 64 * h0:64 * (h0 + 1)],
                            psl[:, 0:512],
                            start=st0, stop=sp0, tile_position=(0, 0))
                        nc.tensor.matmul(
                            ctx_[64:128, 512 * half:512 * (half + 1)],
                            vx[tt][:, 64 * h1:64 * (h1 + 1)],
                            psl[:, 512:1024],
                            start=st0, stop=sp0, tile_position=(0, 64))

                def evac_norm(st):
                    ctx_, pp_ = st[0], st[1]
                    # single evacuation frees the ctx banks; reciprocal+mult
                    # run afterwards from SBUF in DVE slack.  The reciprocal
                    # writes to partitions r0:r0+32 so the multiply's three
                    # operands share a base partition.
                    nc.vector.tensor_copy(ctxU[pp_][:], ctx_[:])
                    for r0 in (0, 64):
                        # cross-base copy aligns the replicated denominators
                        # with the v rows; recip + mult then run same-base.
                        # (reciprocal_approx_fast miscomputes at base
                        # partition 64, so the h1 band uses the standard op.)
                        nc.vector.tensor_copy(
                            recU[r0:r0 + 32, :], ctxU[pp_][r0 + 32:r0 + 64, :])
                        if r0 == 0:
                            nc.vector.reciprocal_approx_fast(
                                recV[r0:r0 + 32, :], recU[r0:r0 + 32, :])
                        else:
                            nc.vector.reciprocal(
                                recV[r0:r0 + 32, :], recU[r0:r0 + 32, :])
                        nc.vector.tensor_tensor(
                            ctxN[pp_][r0:r0 + 32, :], ctxU[pp_][r0:r0 + 32, :],
                            recV[r0:r0 + 32, :], ALU.mult)

                g = 0
                for p in range(4):
                    gblk = p // 2
                    b0 = 32 * ((2 * p) % 4)
                    b1 = b0 + 32
                    ctx = ps2.tile([128, S], F32, tag="ctx", name="ctx", bufs=1)
                    for t in range(NT):
                        eb = ebs[(p, t)]
                        kt = kT[2 * gblk + t // 4]
                        tc4 = 128 * (t % 4)
                        a = (2 * g) % 3
                        b_ = (2 * g + 1) % 3
                        sa = scs3[:, 1024 * a:1024 * (a + 1)]
                        sb = scs3[:, 1024 * b_:1024 * (b_ + 1)]
                        # inject raw bias into PSUM (sets has_written)
                        for sl, off in ((sa, 0), (sb, 1024)):
                            for hh in range(2):
                                nc.tensor.matmul(
                                    sl[:, 512 * hh:512 * (hh + 1)],
                                    identb,
                                    eb[:, off + 512 * hh:off + 512 * (hh + 1)],
                                    start=True, stop=False)
                        # QK^T accumulates on top; half 0 fully first so its
                        # exp never waits on half 1's operands; the two bands
                        # of a half run concurrently in the PE array.
                        for half, sl in ((0, sa), (1, sb)):
                            for bnd, hh in ((b0, 0), (b1, 1)):
                                nc.tensor.matmul(
                                    sl[:, 512 * hh:512 * (hh + 1)],
                                    kt[bnd:bnd + 32, tc4:tc4 + 128],
                                    qT[2 * gblk + half][bnd:bnd + 32, :],
                                    start=False, stop=True,
                                    tile_position=(bnd, 0))
                        X = 2048 * (g % 2)
                        nc.scalar.activation(pts[:, X:X + 1024], sa, AF.Exp)
                        nc.scalar.activation(pts[:, X + 1024:X + 2048], sb, AF.Exp)
                        if prev is not None:
                            pv_emit(prev)
                            if prev[2] == NT - 1:
                                evac_norm(prev)
                        prev = (ctx, p, t, X)
                        g += 1
                pv_emit(prev)
                evac_norm(prev)

            # ====== Phase 3+4: out-proj, LN2, FFN — pipelined per s-half ==
            with tc.tile_pool(name="work3", bufs=4) as wk3, \
                 tc.tile_pool(name="ps34", bufs=1, space="PSUM") as ps34:
                ht = [None] * NT
                # out-proj + residual for all tiles (first FFN-half weights
                # may still be in flight on the second DMA queue otherwise)
                for i in range(NT):
                    pa = ps34.tile([128, D], F32, tag="pa", name="pa", bufs=2)
                    for p in range(4):
                        nc.tensor.matmul(
                            pa[:],
                            ctxN[p][:, 128 * i:128 * (i + 1)],
                            wpk[:, OFF_WO + 256 * p:OFF_WO + 256 * (p + 1)],
                            start=(p == 0), stop=(p == 3))
                    if has_bo:
                        h0t = wk3.tile([128, D], F32, tag="hta", name="hta", bufs=2)
                        nc.vector.tensor_tensor(h0t[:], pa[:], x_res[i][:], ALU.add)
                        h = wk3.tile([128, D], F32, tag="ht", name="ht", bufs=8)
                        nc.vector.tensor_tensor(h[:], h0t[:], bob[:], ALU.add)
                    else:
                        h = wk3.tile([128, D], F32, tag="ht", name="ht", bufs=8)
                        nc.vector.tensor_tensor(h[:], pa[:], x_res[i][:], ALU.add)
                    ht[i] = h

                def ln2_half(half):
                    i0 = 4 * half
                    for i in range(i0, i0 + 4):
                        stats = wk3.tile([128, 6], F32, tag="lnstats2", name="lnstats2")
                        nc.vector.bn_stats(stats[:], ht[i][:])
                        nc.vector.bn_aggr(aggr2[:, 2 * i:2 * i + 2], stats[:])
                    rsqrt_batch(aggr2, rstd2, 8 * half, 8)
                    for i in range(i0, i0 + 4):
                        mean = aggr2[:, 2 * i:2 * i + 1]
                        rstd = rstd2[:, 2 * i + 1:2 * i + 2]
                        if aff2:
                            nc.vector.tensor_scalar(
                                yn[i][:], ht[i][:], mean, rstd, ALU.subtract, ALU.mult)
                            nc.gpsimd.tensor_copy(ybf[i][:], yn[i][:])
                            tmp = wk3.tile([128, D], F32, tag="afftmp2", name="afftmp2")
                            nc.vector.tensor_tensor(tmp[:], yn[i][:], g2b[:], ALU.mult)
                            nc.vector.tensor_tensor(y_res[i][:], tmp[:], be2b[:], ALU.add)
                        else:
                            nc.vector.tensor_scalar(
                                ybf[i][:], ht[i][:], mean, rstd, ALU.subtract, ALU.mult)

                def ytrans_half(half):
                    for j in range(2):
                        tpb = ps34.tile([128, 512], BF16, tag="tpb", name="tpb", bufs=1)
                        for di in range(4):
                            i = 4 * half + di
                            nc.tensor.transpose(
                                tpb[:, 128 * di:128 * (di + 1)],
                                ybf[i][:, 128 * j:128 * (j + 1)], identb)
                        nc.scalar.copy(
                            ynT[j][:, 512 * half:512 * (half + 1)], tpb[:])

                pf2 = {}

                def ff2_emit(j, half):
                    for mo in range(2):
                        nc.tensor.matmul(
                            pf2[mo][:],
                            wpk[:, OFF_W2 + 256 * j + 128 * mo:
                                OFF_W2 + 256 * j + 128 * (mo + 1)],
                            f1T[j][:, 512 * half:512 * (half + 1)],
                            start=(j == 0), stop=(j == 7))

                def ffn_half(half):
                    pf2[0] = ps34.tile([128, 512], F32, tag="pf2a", name="pf2a", bufs=1)
                    pf2[1] = ps34.tile([128, 512], F32, tag="pf2b", name="pf2b", bufs=1)
                    for j in range(8):
                        pf = ps34.tile([128, 512], F32, tag="pf1", name="pf1", bufs=2)
                        for k in range(2):
                            nc.tensor.matmul(
                                pf[:],
                                wpk[:, OFF_W1 + 1024 * k + 128 * j:
                                    OFF_W1 + 1024 * k + 128 * (j + 1)],
                                ynT[k][:, 512 * half:512 * (half + 1)],
                                start=(k == 0), stop=(k == 1))
                        dst = f1T[j][:, 512 * half:512 * (half + 1)]
                        if has_b1:
                            nc.scalar.activation(dst, pf[:], AF.Relu,
                                                 bias=b1c[:, j:j + 1])
                        else:
                            nc.scalar.activation(dst, pf[:], AF.Relu, bias=0.0)
                        if j > 0:
                            ff2_emit(j - 1, half)
                    ff2_emit(7, half)
                    for mo in range(2):
                        dst = f2T[mo][:, 512 * half:512 * (half + 1)]
                        if has_b2:
                            nc.vector.tensor_scalar_add(dst, pf2[mo][:],
                                                        b2c[:, mo:mo + 1])
                        else:
                            nc.scalar.copy(dst, pf2[mo][:])

                def out_half(half):
                    # transpose back + final residual into packed out buffer
                    for i in range(4 * half, 4 * half + 4):
                        tpn = ps34.tile([128, D], BF16, tag="tpn", name="tpn", bufs=1)
                        for j in range(2):
                            nc.tensor.transpose(
                                tpn[:, 128 * j:128 * (j + 1)],
                                f2T[j][:, 128 * i:128 * (i + 1)], identb)
                        nc.vector.tensor_tensor(
                            outbuf[:, D * i:D * (i + 1)], tpn[:], y_res[i][:],
                            ALU.add)
                    nc.scalar.dma_start(
                        out_d.ap().rearrange("(p i) d -> p i d", i=NT)
                            [:, 4 * half:4 * half + 4, :],
                        outbuf[:, 1024 * half:1024 * (half + 1)]
                            .rearrange("p (i d) -> p i d", i=4))

                ln2_half(0)
                ytrans_half(0)
                ln2_half(1)
                ffn_half(0)
                ytrans_half(1)
                out_half(0)
                ffn_half(1)
                out_half(1)

                if dbg:
                    for nm, tl in [
                            ("dbg_xbf0", xbf[0]), ("dbg_xnT0", xnT[0]),
                            ("dbg_qT0", qT[0]), ("dbg_kT0", kT[0]),
                            ("dbg_vx0", vx[0]), ("dbg_ctxU0", ctxU[0]),
                            ("dbg_ctxN0", ctxN[0]), ("dbg_ybf0", ybf[0]),
                            ("dbg_rstd1", rstd1), ("dbg_f1T0", f1T[0]), ("dbg_recU", recU), ("dbg_recV", recV),
                            ("dbg_pts0", None)]:
                        if nm == "dbg_pts0":
                            nc.sync.dma_start(dbg_d[nm].ap(), pts[:, 0:2048])
                        else:
                            nc.sync.dma_start(dbg_d[nm].ap(), tl[:])

    nc.compile()
    return nc


def _prep_host(src, bias, in_proj_w, in_proj_b, out_w, out_b,
               w1, b1, w2, b2, g1, be1, g2, be2):
    f = np.float32
    g1 = np.asarray(g1, f); be1 = np.asarray(be1, f)
    g2 = np.asarray(g2, f); be2 = np.asarray(be2, f)
    in_proj_w = np.asarray(in_proj_w, f); in_proj_b = np.asarray(in_proj_b, f)
    out_w = np.asarray(out_w, f); out_b = np.asarray(out_b, f)
    w1 = np.asarray(w1, f); b1 = np.asarray(b1, f)
    w2 = np.asarray(w2, f); b2 = np.asarray(b2, f)

    winG = in_proj_w * g1[None, :]
    binG = in_proj_w @ be1 + in_proj_b
    scale = HD ** -0.5
    winG[0:D] *= scale
    binG[0:D] *= scale
    wqkT = np.ascontiguousarray(winG[0:2 * D].T)               # [D, 2D]
    bqk = binG[0:2 * D]                                        # [2D]
    wv = winG[2 * D:3 * D]                                     # [D_v, D]
    bv = binG[2 * D:3 * D]
    wvd = np.ascontiguousarray(wv.T)                           # [D, D] dense
    w1G = w1 * g2[None, :]
    b1p = w1 @ be2 + b1

    flags = (
        bool(np.any(g1 != 1.0) or np.any(be1 != 0.0)),
        bool(np.any(g2 != 1.0) or np.any(be2 != 0.0)),
        bool(np.any(bqk != 0.0)),
        bool(np.any(out_b != 0.0)),
        bool(np.any(b1p != 0.0)),
        bool(np.any(b2 != 0.0)),
        bool(np.any(bv != 0.0)),
    )
    aff1, aff2, has_bqk, has_bo, has_b1, has_b2, has_bv = flags

    # ---- weight pack ----
    wpk = np.zeros((128, WPK_COLS), f)
    wpk[:, OFF_ID:OFF_ID + 128] = np.eye(128, dtype=f)
    for k in range(2):
        wpk[:, OFF_QK + 512 * k:OFF_QK + 512 * (k + 1)] = wqkT[128 * k:128 * (k + 1)]
        wpk[:, OFF_WV + 256 * k:OFF_WV + 256 * (k + 1)] = wvd[128 * k:128 * (k + 1)]
        wpk[:, OFF_W1 + 1024 * k:OFF_W1 + 1024 * (k + 1)] = \
            np.ascontiguousarray(w1G.T)[128 * k:128 * (k + 1)]
    woT = np.ascontiguousarray(out_w.T)                        # [D, D]
    for p in range(4):
        blk = np.zeros((128, D), f)
        blk[0:32] = woT[64 * p:64 * p + 32]        # head 2p
        blk[64:96] = woT[64 * p + 32:64 * p + 64]  # head 2p+1
        wpk[:, OFF_WO + 256 * p:OFF_WO + 256 * (p + 1)] = blk
    w2T = np.ascontiguousarray(w2.T)                           # [DFF, D]
    for k in range(8):
        wpk[:, OFF_W2 + 256 * k:OFF_W2 + 256 * (k + 1)] = w2T[128 * k:128 * (k + 1)]

    # ---- bias pack: raw bias^T bf16, one chunk per (pair, t-tile) ----
    # on-chip layout: seq position s lives at (tile i = s%8, partition c = s//8)
    # scores^T chunk (pair, tt): rows tp -> t = 8*tp + tt,
    # cols = [h0 shalf0 | h1 shalf0 | h0 shalf1 | h1 shalf1], each 512 wide;
    # within a half, col index = 128*i4 + c -> s = 8*c + (4*half + i4).
    src = np.asarray(src, f)
    bias = np.asarray(bias, f)
    bT = bias.transpose(0, 1, 3, 2)                # [B, H, t, s]
    x = bT.reshape(B, 4, 2, 128, 8, 128, 2, 4)     # [b,p,hh,tp,tt,c,half,i4]
    x = x.transpose(0, 1, 4, 3, 6, 2, 7, 5)        # [b,p,tt,tp,half,hh,i4,c]
    biasPK = np.ascontiguousarray(x.reshape(B, 4, 8, 128, 2048)).astype(bf16)

    common = {"wpk": wpk.astype(bf16)}
    if has_bv:
        bvbt = np.zeros((128, D), f)
        for h in range(H):
            bvbt[:, 32 * h:32 * (h + 1)] = bv[32 * h:32 * (h + 1)]
        common["bvb"] = bvbt
    if has_bqk:
        common["bqk"] = np.ascontiguousarray(bqk.reshape(4, 128).T)
    if aff1:
        common["g1b"] = np.broadcast_to(g1, (128, D)).copy()
        common["be1b"] = np.broadcast_to(be1, (128, D)).copy()
    if aff2:
        common["g2b"] = np.broadcast_to(g2, (128, D)).copy()
        common["be2b"] = np.broadcast_to(be2, (128, D)).copy()
    if has_bo:
        common["bob"] = np.broadcast_to(out_b, (128, D)).copy()
    if has_b1:
        common["b1c"] = np.ascontiguousarray(b1p.reshape(DFF // 128, 128).T)
    if has_b2:
        common["b2c"] = np.ascontiguousarray(b2.reshape(D // 128, 128).T)

    in_maps = []
    for b in range(N_CORES):
        m = dict(common)
        m["src"] = np.ascontiguousarray(src[:, b, :])
        m["biasPK"] = biasPK[b]
        in_maps.append(m)
    return flags, in_maps


def kernel(**inputs):
    _install_axon_hooks_shim()
    flags, in_maps = _prep_host(
        inputs["src"], inputs["bias"], inputs["in_proj_w"], inputs["in_proj_b"],
        inputs["out_w"], inputs["out_b"], inputs["w1"], inputs["b1"],
        inputs["w2"], inputs["b2"], inputs["g1"], inputs["be1"],
        inputs["g2"], inputs["be2"])
    if flags not in _CACHE:
        _CACHE[flags] = _build(flags)
    nc = _CACHE[flags]
    res = run_bass_kernel_spmd(nc, in_maps, core_ids=list(range(N_CORES)))
    out = np.empty((S, B, D), np.float32)
    for b in range(N_CORES):
        out[:, b, :] = res.results[b]["out"]
    return out


# revision 28
# speedup vs baseline: 1.8447x; 1.8447x over previous
"""Trainium2 Bass kernel for a biased transformer encoder layer.

Full (unsharded) inputs -> full output. Data-parallel over batch B across 8
NeuronCores (one batch element per core).

v2 design points (on top of the v1 identity-injection design):
- Single ACT table set (`exp_and_others`): only Exp / Relu / Copy / Identity
  run on ScalarE.  LayerNorm rsqrt is a batched DVE bit-trick + 2 Newton
  steps; softmax denominators use DVE reciprocal_approx_fast per pair.
  This removes all mid-kernel ACT_TABLE_LOADs and the serial denominator
  stall that let the PE go HAM-cold before the FFN phase.
- exp() stays at [128,1024] per call: with the 3-slot PSUM score ring the
  next iteration's bias-injection reuses the slot freed by the FIRST of
  the two exp calls, so finer exp granularity is what keeps the PE from
  stalling on ScalarE.
- s/t are laid out as seq = 8*partition + tile so src load and output
  store are single DMAs with 8KB/4KB contiguous per-partition lines
  (the bias host-pack absorbs the permutation).
- Output is stored per s-half on the second HWDGE queue (scalar engine)
  so the store overlaps the FFN of the second half.
- PSUM evacuations (x^T, q/k^T, y^T, f2) and the FFN relu run on ScalarE
  (Copy/Relu are in the exp table set); LayerNorm stats/apply stay on DVE.
"""

import numpy as np
import ml_dtypes

import concourse.mybir as mybir
import concourse.tile as tile
from concourse import bacc
from concourse.bass_utils import run_bass_kernel_spmd

# ---- problem constants (hardcoded per contract) ----
S = 1024
B = 8
D = 256
H = 8
HD = D // H          # 32
DFF = 1024
EPS = 1e-5
N_CORES = 8
NT = S // 128        # 8 s-tiles / t-tiles

F32 = mybir.dt.float32
BF16 = mybir.dt.bfloat16
I32 = mybir.dt.int32
bf16 = ml_dtypes.bfloat16
AF = mybir.ActivationFunctionType
ALU = mybir.AluOpType

# weight-pack column offsets (shared by _build and _prep_host)
OFF_ID = 0                      # identity          [128, 128]
OFF_QK = OFF_ID + 128           # wqkT  2x[128,512]
OFF_WV = OFF_QK + 1024          # wv dense 2x[128,256]
OFF_WO = OFF_WV + 512           # woPK  4x[128,256] (zero-padded pair blocks)
OFF_W1 = OFF_WO + 1024          # w1T   2x[128,1024]
OFF_W2 = OFF_W1 + 2048          # w2T   8x[128,256]
WPK_COLS = OFF_W2 + 2048        # 6784
WPK_A = OFF_W1                  # phase-1/2 weights (first DMA chunk)

RSQRT_MAGIC1 = 0x5F3759DF + 1   # seed = (~(i>>1)) + MAGIC+1 == MAGIC - (i>>1)

_CACHE = {}


def _install_axon_hooks_shim():
    """Make `trace=True` degrade gracefully if antenv.axon_hooks is missing."""
    import sys, types
    try:
        import antenv  # noqa
    except ImportError:
        return
    if "antenv.axon_hooks" in sys.modules:
        return
    try:
        import antenv.axon_hooks  # noqa
    except ImportError:
        import antenv
        mod = types.ModuleType("antenv.axon_hooks")
        _hook = [None]
        mod.set_axon_ntff_profile_hook = lambda h: _hook.__setitem__(0, h)
        mod.get_axon_ntff_profile_hook = lambda: _hook[0]
        sys.modules["antenv.axon_hooks"] = mod
        antenv.axon_hooks = mod


def _build(flags):
    """Build the Bass program (shared by all 8 cores, SPMD)."""
    (aff1, aff2, has_bqk, has_bo, has_b1, has_b2, has_bv) = flags
    nc = bacc.Bacc("TRN2", debug=False, num_devices=N_CORES, enable_asserts=True)

    # ---- DRAM tensors (per-core inputs) ----
    src_d = nc.dram_tensor("src", [S, D], F32, kind="ExternalInput")
    expb_d = nc.dram_tensor("biasPK", [4, 8, 128, 2048], BF16, kind="ExternalInput")
    wpk_d = nc.dram_tensor("wpk", [128, WPK_COLS], BF16, kind="ExternalInput")
    if has_bv:
        bvb_d = nc.dram_tensor("bvb", [128, D], F32, kind="ExternalInput")
    if has_bqk:
        bqk_d = nc.dram_tensor("bqk", [128, 4], F32, kind="ExternalInput")
    if aff1:
        g1b_d = nc.dram_tensor("g1b", [128, D], F32, kind="ExternalInput")
        be1b_d = nc.dram_tensor("be1b", [128, D], F32, kind="ExternalInput")
    if aff2:
        g2b_d = nc.dram_tensor("g2b", [128, D], F32, kind="ExternalInput")
        be2b_d = nc.dram_tensor("be2b", [128, D], F32, kind="ExternalInput")
    if has_bo:
        bob_d = nc.dram_tensor("bob", [128, D], F32, kind="ExternalInput")
    if has_b1:
        b1c_d = nc.dram_tensor("b1c", [128, DFF // 128], F32, kind="ExternalInput")
    if has_b2:
        b2c_d = nc.dram_tensor("b2c", [128, D // 128], F32, kind="ExternalInput")
    out_d = nc.dram_tensor("out", [S, D], F32, kind="ExternalOutput")
    import os
    dbg = os.environ.get("KBG_DEBUG", "")
    dbg_d = {}
    if dbg:
        for nm, shape, dt in [
                ("dbg_xbf0", [128, D], BF16), ("dbg_xnT0", [128, S], BF16),
                ("dbg_qT0", [128, 512], BF16), ("dbg_kT0", [128, 512], BF16),
                ("dbg_vx0", [128, 2 * D], BF16),
                ("dbg_ctxU0", [128, S], F32), ("dbg_ctxN0", [128, S], BF16),
                ("dbg_ybf0", [128, D], BF16), ("dbg_rstd1", [128, 2 * NT], F32),
                ("dbg_f1T0", [128, S], BF16), ("dbg_recU", [128, S], F32), ("dbg_recV", [128, S], F32)]:
            dbg_d[nm] = nc.dram_tensor(nm, shape, dt, kind="ExternalOutput")

    with tile.TileContext(nc, trace_sim=True) as tc:
        with tc.tile_pool(name="persist", bufs=1) as pp, \
             tc.tile_pool(name="battn", bufs=16) as battn:
            wpk = pp.tile([128, WPK_COLS], BF16, tag="wpk", name="wpk")
            identb = wpk[:, OFF_ID:OFF_ID + 128]

            if has_bv:
                bvb = pp.tile([128, D], F32, tag="bvb", name="bvb")
            if has_bqk:
                bqk = pp.tile([128, 4], F32, tag="bqk", name="bqk")
            if aff1:
                g1b = pp.tile([128, D], F32, tag="g1b", name="g1b")
                be1b = pp.tile([128, D], F32, tag="be1b", name="be1b")
            if aff2:
                g2b = pp.tile([128, D], F32, tag="g2b", name="g2b")
                be2b = pp.tile([128, D], F32, tag="be2b", name="be2b")
            if has_bo:
                bob = pp.tile([128, D], F32, tag="bob", name="bob")
            if has_b1:
                b1c = pp.tile([128, DFF // 128], F32, tag="b1c", name="b1c")
            if has_b2:
                b2c = pp.tile([128, D // 128], F32, tag="b2c", name="b2c")

            # ---- persistent activations ----
            srcall = pp.tile([128, 8 * D], F32, tag="srcall", name="srcall")
            xbf = [pp.tile([128, D], BF16, tag=f"xbf{i}", name=f"xbf{i}") for i in range(NT)]
            x_res = xbf
            if aff1:
                xn = [pp.tile([128, D], F32, tag=f"xn{i}", name=f"xn{i}") for i in range(NT)]
                x_res = [pp.tile([128, D], F32, tag=f"xr{i}", name=f"xr{i}") for i in range(NT)]
            xnT = [pp.tile([128, S], BF16, tag=f"xnT{k}", name=f"xnT{k}") for k in range(2)]
            qT = [pp.tile([128, 512], BF16, tag=f"qT{k}", name=f"qT{k}") for k in range(4)]
            kT = [pp.tile([128, 512], BF16, tag=f"kT{k}", name=f"kT{k}") for k in range(4)]
            vx = [pp.tile([128, 2 * D], BF16, tag=f"vx{i}", name=f"vx{i}") for i in range(NT)]
            ctxU = [pp.tile([128, S], F32, tag=f"ctxU{p}", name=f"ctxU{p}") for p in range(4)]
            ctxN = [pp.tile([128, S], BF16, tag=f"ctxN{p}", name=f"ctxN{p}") for p in range(4)]
            recU = pp.tile([128, S], F32, tag="recU", name="recU")
            recV = pp.tile([128, S], F32, tag="recV", name="recV")
            ybf = [pp.tile([128, D], BF16, tag=f"ybf{i}", name=f"ybf{i}") for i in range(NT)]
            y_res = ybf
            if aff2:
                yn = [pp.tile([128, D], F32, tag=f"yn{i}", name=f"yn{i}") for i in range(NT)]
                y_res = [pp.tile([128, D], F32, tag=f"yr{i}", name=f"yr{i}") for i in range(NT)]
            ynT = [pp.tile([128, S], BF16, tag=f"ynT{k}", name=f"ynT{k}") for k in range(2)]
            f1T = [pp.tile([128, S], BF16, tag=f"f1T{m}", name=f"f1T{m}") for m in range(8)]
            f2T = [pp.tile([128, S], BF16, tag=f"f2T{m}", name=f"f2T{m}") for m in range(2)]
            outbuf = [pp.tile([128, 4 * D], F32, tag=f"outbuf{h}", name=f"outbuf{h}")
                      for h in range(2)]

            # rsqrt scratch (batched over tiles; [mean,var] interleaved flat —
            # the chain runs over both columns; mean lanes produce garbage
            # that is never consumed)
            aggr1 = pp.tile([128, 2 * NT], F32, tag="aggr1", name="aggr1")
            aggr2 = pp.tile([128, 2 * NT], F32, tag="aggr2", name="aggr2")
            rstd1 = pp.tile([128, 2 * NT], F32, tag="rstd1", name="rstd1")
            rstd2 = pp.tile([128, 2 * NT], F32, tag="rstd2", name="rstd2")
            rs_v = pp.tile([128, 2 * NT], F32, tag="rs_v", name="rs_v")
            rs_sh = pp.tile([128, 2 * NT], I32, tag="rs_sh", name="rs_sh")
            rs_y = pp.tile([128, 2 * NT], F32, tag="rs_y", name="rs_y")
            rs_a = pp.tile([128, 2 * NT], F32, tag="rs_a", name="rs_a")
            rs_b = pp.tile([128, 2 * NT], F32, tag="rs_b", name="rs_b")
            rs_y2 = pp.tile([128, 2 * NT], F32, tag="rs_y2", name="rs_y2")
            dum = pp.tile([128, 1], F32, tag="dum", name="dum")

            # table preload: dummy exp is the first ScalarE activation, so the
            # single exp_and_others ACT_TABLE_LOAD overlaps the initial DMAs.
            nc.gpsimd.memset(dum[:], 0.0)
            nc.scalar.activation(dum[:], dum[:], AF.Exp)

            # vx: memset to 1.0; v columns overwritten later (surviving 1.0
            # columns become the softmax-denominator "ones" blocks).
            for i in range(NT):
                nc.gpsimd.memset(vx[i][:], 1.0)
            # ctxN den rows must be 0.0 so the (zero-padded) out-proj blocks
            # never meet NaN garbage; v rows are overwritten per pair.
            for p in range(4):
                nc.gpsimd.memset(ctxN[p][:], 0.0)

            # ---- DMAs: src first (gates phase 1), then phase-1/2 weights,
            # then the full bias stream (pool-gated), FFN weights on the
            # second HWDGE queue so they don't block the bias stream.
            nc.sync.dma_start(
                srcall[:].rearrange("p (i d) -> p i d", i=NT),
                src_d.ap().rearrange("(p i) d -> p i d", i=NT))
            nc.sync.dma_start(wpk[:, 0:WPK_A], wpk_d.ap()[:, 0:WPK_A])
            nc.scalar.dma_start(wpk[:, WPK_A:WPK_COLS], wpk_d.ap()[:, WPK_A:WPK_COLS])
            if has_bv:
                nc.scalar.dma_start(bvb[:], bvb_d.ap())
            if has_bqk:
                nc.scalar.dma_start(bqk[:], bqk_d.ap())
            if aff1:
                nc.scalar.dma_start(g1b[:], g1b_d.ap())
                nc.scalar.dma_start(be1b[:], be1b_d.ap())
            if aff2:
                nc.scalar.dma_start(g2b[:], g2b_d.ap())
                nc.scalar.dma_start(be2b[:], be2b_d.ap())
            if has_bo:
                nc.scalar.dma_start(bob[:], bob_d.ap())
            if has_b1:
                nc.scalar.dma_start(b1c[:], b1c_d.ap())
            if has_b2:
                nc.scalar.dma_start(b2c[:], b2c_d.ap())
            ebs = {}
            for p in range(4):
                for t in range(NT):
                    eb = battn.tile([128, 2048], BF16, tag="eb", name="eb")
                    nc.sync.dma_start(eb[:], expb_d.ap()[p, t])
                    ebs[(p, t)] = eb

            def rsqrt_batch(aggr, rstd_out, c0, n2):
                """rstd_out[:, c] = 1/sqrt(aggr[:, c] + EPS) for c in
                [c0, c0+n2).  Runs over [mean,var] interleaved columns; only
                odd (var) lanes are consumed.  Quake-III seed + 2 Newton
                steps, all on DVE (no ACT tables)."""
                sl = (slice(None), slice(c0, c0 + n2))
                v = rs_v[sl]
                nc.vector.tensor_scalar_add(v, aggr[sl], EPS)
                nc.vector.tensor_scalar(
                    rs_sh[sl], v.bitcast(I32), 1, -1,
                    ALU.logical_shift_right, ALU.bitwise_xor)
                nc.vector.tensor_scalar_add(
                    rs_y[sl].bitcast(I32), rs_sh[sl], RSQRT_MAGIC1)
                y = rs_y[sl]
                for it in range(2):
                    yo = (rs_y2 if it == 0 else rstd_out)[sl]
                    nc.vector.tensor_tensor(rs_a[sl], y, y, ALU.mult)
                    nc.vector.tensor_tensor(rs_b[sl], rs_a[sl], v, ALU.mult)
                    nc.vector.tensor_scalar(
                        rs_a[sl], rs_b[sl], -0.5, 1.5, ALU.mult, ALU.add)
                    nc.vector.tensor_tensor(yo, rs_a[sl], y, ALU.mult)
                    y = yo

            # ================= Phase 1: LN1, transposes, v, qkT =============
            with tc.tile_pool(name="work1", bufs=4) as wk, \
                 tc.tile_pool(name="ps1", bufs=1, space="PSUM") as ps1:

                def ln1_apply(i):
                    st = srcall[:, D * i:D * (i + 1)]
                    mean = aggr1[:, 2 * i:2 * i + 1]
                    rstd = rstd1[:, 2 * i + 1:2 * i + 2]
                    if aff1:
                        nc.vector.tensor_scalar(
                            xn[i][:], st, mean, rstd, ALU.subtract, ALU.mult)
                        nc.gpsimd.tensor_copy(xbf[i][:], xn[i][:])
                        tmp = wk.tile([128, D], F32, tag="afftmp", name="afftmp")
                        nc.vector.tensor_tensor(tmp[:], xn[i][:], g1b[:], ALU.mult)
                        nc.vector.tensor_tensor(x_res[i][:], tmp[:], be1b[:], ALU.add)
                    else:
                        nc.vector.tensor_scalar(
                            xbf[i][:], st, mean, rstd, ALU.subtract, ALU.mult)

                # batched LN1 stats per s-half, then apply + transpose + v-proj
                for half in range(2):
                    i0 = 4 * half
                    for i in range(i0, i0 + 4):
                        st = srcall[:, D * i:D * (i + 1)]
                        stats = wk.tile([128, 6], F32, tag="lnstats", name="lnstats")
                        nc.vector.bn_stats(stats[:], st)
                        nc.vector.bn_aggr(aggr1[:, 2 * i:2 * i + 2], stats[:])
                    rsqrt_batch(aggr1, rstd1, 2 * i0, 8)
                    for i in range(i0, i0 + 4):
                        ln1_apply(i)
                        # transpose s-tile into xnT columns (both d-blocks)
                        tp = ps1.tile([128, 256], BF16, tag="tp", name="tp", bufs=2)
                        for j in range(2):
                            nc.tensor.transpose(
                                tp[:, 128 * j:128 * (j + 1)],
                                xbf[i][:, 128 * j:128 * (j + 1)], identb)
                        for j in range(2):
                            nc.scalar.copy(
                                xnT[j][:, 128 * i:128 * (i + 1)],
                                tp[:, 128 * j:128 * (j + 1)])
                        # dense v projection for this tile
                        pv = ps1.tile([128, D], F32, tag="pv", name="pv", bufs=2)
                        for k in range(2):
                            nc.tensor.matmul(
                                pv[:],
                                xnT[k][:, 128 * i:128 * (i + 1)],
                                wpk[:, OFF_WV + 256 * k:OFF_WV + 256 * (k + 1)],
                                start=(k == 0), stop=(k == 1))
                        vdst = vx[i][:].rearrange("p (h c) -> p h c", h=H)[:, :, 0:HD]
                        vsrc = pv[:].rearrange("p (h c) -> p h c", h=H)
                        if has_bv:
                            bsrc = bvb[:].rearrange("p (h c) -> p h c", h=H)
                            nc.vector.tensor_tensor(vdst, vsrc, bsrc, ALU.add)
                        else:
                            nc.vector.tensor_copy(vdst, vsrc)
                    # qkT for the completed s-half
                    for m in (0, 2, 1, 3):  # q0,k0 first
                        dstT = (qT[2 * m + half] if m < 2
                                else kT[2 * (m - 2) + half])
                        pq = ps1.tile([128, 512], F32, tag="pq", name="pq", bufs=2)
                        for k in range(2):
                            nc.tensor.matmul(
                                pq[:],
                                wpk[:, OFF_QK + 512 * k + 128 * m:
                                    OFF_QK + 512 * k + 128 * (m + 1)],
                                xnT[k][:, 512 * half:512 * (half + 1)],
                                start=(k == 0), stop=(k == 1))
                        if has_bqk:
                            nc.vector.tensor_scalar_add(dstT[:], pq[:], bqk[:, m:m + 1])
                        elif m < 2:
                            nc.scalar.copy(dstT[:], pq[:])
                        else:
                            nc.vector.tensor_copy(dstT[:], pq[:])

            # ================= Phase 2: attention =================
            # Per (pair, t): raw bias is injected into PSUM by identity
            # matmuls (start=True), QK^T accumulates on top (row-tiled bands,
            # concurrent); exp spans both 1024-col halves in one ScalarE call
            # when the rotation slots are contiguous.  PV is emitted with one
            # iteration of lag so the PE FIFO never blocks on exp.
            with tc.tile_pool(name="ps2", bufs=1, space="PSUM") as ps2, \
                 tc.tile_pool(name="ptp", bufs=4) as ptp:
                prev = None   # deferred PV emission (cross-iteration lag)

                def pv_emit(st):
                    ctx_, pp_, tt, pts_ = st
                    st0, sp0 = (tt == 0), (tt == NT - 1)
                    h0, h1 = 2 * pp_, 2 * pp_ + 1
                    for half in range(2):
                        psl = pts_[half]
                        nc.tensor.matmul(
                            ctx_[0:64, 512 * half:512 * (half + 1)],
                            vx[tt][:, 64 * h0:64 * (h0 + 1)],
                            psl[:, 0:512],
                            start=st0, stop=sp0, tile_position=(0, 0))
                        nc.tensor.matmul(
                            ctx_[64:128, 512 * half:512 * (half + 1)],
                            vx[tt][:, 64 * h1:64 * (h1 + 1)],
                            psl[:, 512:1024],
                            start=st0, stop=sp0, tile_position=(0, 64))

                def evac_norm(st):
                    ctx_, pp_ = st[0], st[1]
                    # single evacuation frees the ctx banks; reciprocal+mult
                    # run afterwards from SBUF in DVE slack.  The reciprocal
                    # writes to partitions r0:r0+32 so the multiply's three
                    # operands share a base partition.
                    nc.vector.tensor_copy(ctxU[pp_][:], ctx_[:])
                    for r0 in (0, 64):
                        # cross-base copy aligns the replicated denominators
                        # with the v rows; recip + mult then run same-base.
                        # (reciprocal_approx_fast miscomputes at base
                        # partition 64, so the h1 band uses the standard op.)
                        nc.vector.tensor_copy(
                            recU[r0:r0 + 32, :], ctxU[pp_][r0 + 32:r0 + 64, :])
                        if r0 == 0:
                            nc.vector.reciprocal_approx_fast(
                                recV[r0:r0 + 32, :], recU[r0:r0 + 32, :])
                        else:
                            nc.vector.reciprocal(
                                recV[r0:r0 + 32, :], recU[r0:r0 + 32, :])
                        nc.vector.tensor_tensor(
                            ctxN[pp_][r0:r0 + 32, :], ctxU[pp_][r0:r0 + 32, :],
                            recV[r0:r0 + 32, :], ALU.mult)

                g = 0
                for p in range(4):
                    gblk = p // 2
                    b0 = 32 * ((2 * p) % 4)
                    b1 = b0 + 32
                    ctx = ps2.tile([128, S], F32, tag="ctx", name="ctx", bufs=1)
                    for t in range(NT):
                        eb = ebs[(p, t)]
                        kt = kT[2 * gblk + t // 4]
                        tc4 = 128 * (t % 4)
                        scs = [ps2.tile([128, 1024], F32, tag="sc", name="sc",
                                        bufs=3) for _ in range(2)]
                        # inject raw bias into PSUM (sets has_written)
                        for half, sl in ((0, scs[0]), (1, scs[1])):
                            for hh in range(2):
                                nc.tensor.matmul(
                                    sl[:, 512 * hh:512 * (hh + 1)],
                                    identb,
                                    eb[:, 1024 * half + 512 * hh:
                                        1024 * half + 512 * (hh + 1)],
                                    start=True, stop=False)
                        # QK^T accumulates on top; half 0 fully first so its
                        # exp never waits on half 1's operands; the two bands
                        # of a half run concurrently in the PE array.
                        for half, sl in ((0, scs[0]), (1, scs[1])):
                            for bnd, hh in ((b0, 0), (b1, 1)):
                                nc.tensor.matmul(
                                    sl[:, 512 * hh:512 * (hh + 1)],
                                    kt[bnd:bnd + 32, tc4:tc4 + 128],
                                    qT[2 * gblk + half][bnd:bnd + 32, :],
                                    start=False, stop=True,
                                    tile_position=(bnd, 0))
                        pts_ = []
                        for half in range(2):
                            pt = ptp.tile([128, 1024], BF16, tag="pt",
                                          name="pt", bufs=4)
                            nc.scalar.activation(pt[:], scs[half][:], AF.Exp)
                            pts_.append(pt)
                        if prev is not None:
                            pv_emit(prev)
                            if prev[2] == NT - 1:
                                evac_norm(prev)
                        prev = (ctx, p, t, pts_)
                        g += 1
                pv_emit(prev)
                evac_norm(prev)

            # ====== Phase 3+4: out-proj, LN2, FFN — pipelined per s-half ==
            with tc.tile_pool(name="work3", bufs=4) as wk3, \
                 tc.tile_pool(name="ps34", bufs=1, space="PSUM") as ps34:
                ht = [None] * NT
                # out-proj + residual for all tiles (first FFN-half weights
                # may still be in flight on the second DMA queue otherwise)
                for i in range(NT):
                    pa = ps34.tile([128, D], F32, tag="pa", name="pa", bufs=2)
                    for p in range(4):
                        nc.tensor.matmul(
                            pa[:],
                            ctxN[p][:, 128 * i:128 * (i + 1)],
                            wpk[:, OFF_WO + 256 * p:OFF_WO + 256 * (p + 1)],
                            start=(p == 0), stop=(p == 3))
                    if has_bo:
                        h0t = wk3.tile([128, D], F32, tag="hta", name="hta", bufs=2)
                        nc.vector.tensor_tensor(h0t[:], pa[:], x_res[i][:], ALU.add)
                        h = wk3.tile([128, D], F32, tag="ht", name="ht", bufs=8)
                        nc.vector.tensor_tensor(h[:], h0t[:], bob[:], ALU.add)
                    else:
                        h = wk3.tile([128, D], F32, tag="ht", name="ht", bufs=8)
                        nc.vector.tensor_tensor(h[:], pa[:], x_res[i][:], ALU.add)
                    ht[i] = h

                def ln2_half(half):
                    i0 = 4 * half
                    for i in range(i0, i0 + 4):
                        stats = wk3.tile([128, 6], F32, tag="lnstats2", name="lnstats2")
                        nc.vector.bn_stats(stats[:], ht[i][:])
                        nc.vector.bn_aggr(aggr2[:, 2 * i:2 * i + 2], stats[:])
                    rsqrt_batch(aggr2, rstd2, 8 * half, 8)
                    for i in range(i0, i0 + 4):
                        mean = aggr2[:, 2 * i:2 * i + 1]
                        rstd = rstd2[:, 2 * i + 1:2 * i + 2]
                        if aff2:
                            nc.vector.tensor_scalar(
                                yn[i][:], ht[i][:], mean, rstd, ALU.subtract, ALU.mult)
                            nc.gpsimd.tensor_copy(ybf[i][:], yn[i][:])
                            tmp = wk3.tile([128, D], F32, tag="afftmp2", name="afftmp2")
                            nc.vector.tensor_tensor(tmp[:], yn[i][:], g2b[:], ALU.mult)
                            nc.vector.tensor_tensor(y_res[i][:], tmp[:], be2b[:], ALU.add)
                        else:
                            nc.vector.tensor_scalar(
                                ybf[i][:], ht[i][:], mean, rstd, ALU.subtract, ALU.mult)

                def ytrans_half(half):
                    for j in range(2):
                        tpb = ps34.tile([128, 512], BF16, tag="tpb", name="tpb", bufs=1)
                        for di in range(4):
                            i = 4 * half + di
                            nc.tensor.transpose(
                                tpb[:, 128 * di:128 * (di + 1)],
                                ybf[i][:, 128 * j:128 * (j + 1)], identb)
                        nc.scalar.copy(
                            ynT[j][:, 512 * half:512 * (half + 1)], tpb[:])

                pf2 = {}

                def ff2_emit(j, half):
                    for mo in range(2):
                        nc.tensor.matmul(
                            pf2[mo][:],
                            wpk[:, OFF_W2 + 256 * j + 128 * mo:
                                OFF_W2 + 256 * j + 128 * (mo + 1)],
                            f1T[j][:, 512 * half:512 * (half + 1)],
                            start=(j == 0), stop=(j == 7))

                def ffn_half(half):
                    pf2[0] = ps34.tile([128, 512], F32, tag="pf2a", name="pf2a", bufs=1)
                    pf2[1] = ps34.tile([128, 512], F32, tag="pf2b", name="pf2b", bufs=1)
                    for j in range(8):
                        pf = ps34.tile([128, 512], F32, tag="pf1", name="pf1", bufs=2)
                        for k in range(2):
                            nc.tensor.matmul(
                                pf[:],
                                wpk[:, OFF_W1 + 1024 * k + 128 * j:
                                    OFF_W1 + 1024 * k + 128 * (j + 1)],
                                ynT[k][:, 512 * half:512 * (half + 1)],
                                start=(k == 0), stop=(k == 1))
                        dst = f1T[j][:, 512 * half:512 * (half + 1)]
                        if has_b1:
                            nc.scalar.activation(dst, pf[:], AF.Relu,
                                                 bias=b1c[:, j:j + 1])
                        else:
                            nc.scalar.activation(dst, pf[:], AF.Relu, bias=0.0)
                        if j > 0:
                            ff2_emit(j - 1, half)
                    ff2_emit(7, half)
                    for mo in range(2):
                        dst = f2T[mo][:, 512 * half:512 * (half + 1)]
                        if has_b2:
                            nc.vector.tensor_scalar_add(dst, pf2[mo][:],
                                                        b2c[:, mo:mo + 1])
                        else:
                            nc.scalar.copy(dst, pf2[mo][:])

                def out_half(half):
                    # transpose back + final residual into packed out buffer
                    for i in range(4 * half, 4 * half + 4):
                        tpn = ps34.tile([128, D], BF16, tag="tpn", name="tpn", bufs=1)
                        for j in range(2):
                            nc.tensor.transpose(
                                tpn[:, 128 * j:128 * (j + 1)],
                                f2T[j][:, 128 * i:128 * (i + 1)], identb)
                        nc.vector.tensor_tensor(
                            outbuf[half][:, D * (i - 4 * half):D * (i - 4 * half + 1)],
                            tpn[:], y_res[i][:], ALU.add)
                    nc.scalar.dma_start(
                        out_d.ap().rearrange("(p i) d -> p i d", i=NT)
                            [:, 4 * half:4 * half + 4, :],
                        outbuf[half][:].rearrange("p (i d) -> p i d", i=4))

                ln2_half(0)
                ytrans_half(0)
                ln2_half(1)
                ffn_half(0)
                ytrans_half(1)
                out_half(0)
                ffn_half(1)
                out_half(1)

                if dbg:
                    for nm, tl in [
                            ("dbg_xbf0", xbf[0]), ("dbg_xnT0", xnT[0]),
                            ("dbg_qT0", qT[0]), ("dbg_kT0", kT[0]),
                            ("dbg_vx0", vx[0]), ("dbg_ctxU0", ctxU[0]),
                            ("dbg_ctxN0", ctxN[0]), ("dbg_ybf0", ybf[0]),
                            ("dbg_rstd1", rstd1), ("dbg_f1T0", f1T[0]),
                            ("dbg_recU", recU), ("dbg_recV", recV)]:
                        nc.sync.dma_start(dbg_d[nm].ap(), tl[:])

    nc.compile()
    return nc


def _prep_host(src, bias, in_proj_w, in_proj_b, out_w, out_b,
               w1, b1, w2, b2, g1, be1, g2, be2):
    f = np.float32
    g1 = np.asarray(g1, f); be1 = np.asarray(be1, f)
    g2 = np.asarray(g2, f); be2 = np.asarray(be2, f)
    in_proj_w = np.asarray(in_proj_w, f); in_proj_b = np.asarray(in_proj_b, f)
    out_w = np.asarray(out_w, f); out_b = np.asarray(out_b, f)
    w1 = np.asarray(w1, f); b1 = np.asarray(b1, f)
    w2 = np.asarray(w2, f); b2 = np.asarray(b2, f)

    winG = in_proj_w * g1[None, :]
    binG = in_proj_w @ be1 + in_proj_b
    scale = HD ** -0.5
    winG[0:D] *= scale
    binG[0:D] *= scale
    wqkT = np.ascontiguousarray(winG[0:2 * D].T)               # [D, 2D]
    bqk = binG[0:2 * D]                                        # [2D]
    wv = winG[2 * D:3 * D]                                     # [D_v, D]
    bv = binG[2 * D:3 * D]
    wvd = np.ascontiguousarray(wv.T)                           # [D, D] dense
    w1G = w1 * g2[None, :]
    b1p = w1 @ be2 + b1

    flags = (
        bool(np.any(g1 != 1.0) or np.any(be1 != 0.0)),
        bool(np.any(g2 != 1.0) or np.any(be2 != 0.0)),
        bool(np.any(bqk != 0.0)),
        bool(np.any(out_b != 0.0)),
        bool(np.any(b1p != 0.0)),
        bool(np.any(b2 != 0.0)),
        bool(np.any(bv != 0.0)),
    )
    aff1, aff2, has_bqk, has_bo, has_b1, has_b2, has_bv = flags

    # ---- weight pack ----
    wpk = np.zeros((128, WPK_COLS), f)
    wpk[:, OFF_ID:OFF_ID + 128] = np.eye(128, dtype=f)
    for k in range(2):
        wpk[:, OFF_QK + 512 * k:OFF_QK + 512 * (k + 1)] = wqkT[128 * k:128 * (k + 1)]
        wpk[:, OFF_WV + 256 * k:OFF_WV + 256 * (k + 1)] = wvd[128 * k:128 * (k + 1)]
        wpk[:, OFF_W1 + 1024 * k:OFF_W1 + 1024 * (k + 1)] = \
            np.ascontiguousarray(w1G.T)[128 * k:128 * (k + 1)]
    woT = np.ascontiguousarray(out_w.T)                        # [D, D]
    for p in range(4):
        blk = np.zeros((128, D), f)
        blk[0:32] = woT[64 * p:64 * p + 32]        # head 2p
        blk[64:96] = woT[64 * p + 32:64 * p + 64]  # head 2p+1
        wpk[:, OFF_WO + 256 * p:OFF_WO + 256 * (p + 1)] = blk
    w2T = np.ascontiguousarray(w2.T)                           # [DFF, D]
    for k in range(8):
        wpk[:, OFF_W2 + 256 * k:OFF_W2 + 256 * (k + 1)] = w2T[128 * k:128 * (k + 1)]

    # ---- bias pack: raw bias^T bf16, one chunk per (pair, t-tile) ----
    # on-chip layout: seq position s lives at (tile i = s%8, partition c = s//8)
    # scores^T chunk (pair, tt): rows tp -> t = 8*tp + tt,
    # cols = [h0 shalf0 | h1 shalf0 | h0 shalf1 | h1 shalf1], each 512 wide;
    # within a half, col index = 128*i4 + c -> s = 8*c + (4*half + i4).
    src = np.asarray(src, f)
    bias = np.asarray(bias, f)
    bT = bias.transpose(0, 1, 3, 2)                # [B, H, t, s]
    x = bT.reshape(B, 4, 2, 128, 8, 128, 2, 4)     # [b,p,hh,tp,tt,c,half,i4]
    x = x.transpose(0, 1, 4, 3, 6, 2, 7, 5)        # [b,p,tt,tp,half,hh,i4,c]
    biasPK = np.ascontiguousarray(x.reshape(B, 4, 8, 128, 2048)).astype(bf16)

    common = {"wpk": wpk.astype(bf16)}
    if has_bv:
        bvbt = np.zeros((128, D), f)
        for h in range(H):
            bvbt[:, 32 * h:32 * (h + 1)] = bv[32 * h:32 * (h + 1)]
        common["bvb"] = bvbt
    if has_bqk:
        common["bqk"] = np.ascontiguousarray(bqk.reshape(4, 128).T)
    if aff1:
        common["g1b"] = np.broadcast_to(g1, (128, D)).copy()
        common["be1b"] = np.broadcast_to(be1, (128, D)).copy()
    if aff2:
        common["g2b"] = np.broadcast_to(g2, (128, D)).copy()
        common["be2b"] = np.broadcast_to(be2, (128, D)).copy()
    if has_bo:
        common["bob"] = np.broadcast_to(out_b, (128, D)).copy()
    if has_b1:
        common["b1c"] = np.ascontiguousarray(b1p.reshape(DFF // 128, 128).T)
    if has_b2:
        common["b2c"] = np.ascontiguousarray(b2.reshape(D // 128, 128).T)

    in_maps = []
    for b in range(N_CORES):
        m = dict(common)
        m["src"] = np.ascontiguousarray(src[:, b, :])
        m["biasPK"] = biasPK[b]
        in_maps.append(m)
    return flags, in_maps


def kernel(**inputs):
    _install_axon_hooks_shim()
    flags, in_maps = _prep_host(
        inputs["src"], inputs["bias"], inputs["in_proj_w"], inputs["in_proj_b"],
        inputs["out_w"], inputs["out_b"], inputs["w1"], inputs["b1"],
        inputs["w2"], inputs["b2"], inputs["g1"], inputs["be1"],
        inputs["g2"], inputs["be2"])
    if flags not in _CACHE:
        _CACHE[flags] = _build(flags)
    nc = _CACHE[flags]
    res = run_bass_kernel_spmd(nc, in_maps, core_ids=list(range(N_CORES)))
    out = np.empty((S, B, D), np.float32)
    for b in range(N_CORES):
        out[:, b, :] = res.results[b]["out"]
    return out
